# revision 21
# baseline (speedup 1.0000x reference)
"""GQA attention (B=2, S=2048, H=2048, 16 Q heads / 4 KV heads, d=128) on
8 TRN2 NeuronCores.

Sharding: core c = (batch b = c//4, kv-group g = c%4). Each core computes
Q/K/V projections and attention for its 4 Q heads of its batch, then four
8-wide AllToAlls (one per local head, issued as soon as that head's
attention finishes, so comm overlaps compute) redistribute attention
outputs head-sharded -> sequence-sharded. Sends are duplicated to both
batch halves; a per-core mask input selects the right half on receive.
Finally each core computes the full-width o_proj for its sequence quarter.

Key scheduling devices (v2):
- A dummy-matmul warmup stream keeps the PE busy from ~9us (while the
  input DMAs stream) so the p-state ramp completes before real work and
  the DMA lead-in is hidden.
- Scores matmuls are emitted in PAIRS into one [128,1024] two-bank PSUM
  tile consumed by a single wide exp on ACT: the activation's fixed
  overhead is amortized so ACT (~1.0us/pair) stays ahead of the PE
  (~1.3us/pair) in the attention inner loop.
- PV accumulation for pair p runs while exp of pair p+1 is in flight
  (one-pair skew), so the PE never waits on the current exp.
- The four per-chunk PV accumulators (with ones-column denominators) are
  PACKED 3+1 into two PSUM banks (one accumulation group per bank,
  start on the bank's first matmul, stop on its last), freeing banks
  for the wide score pairs. PSUM: scores 2x2 + accs 2 + psq 1 + tp 1 = 8.
- hidden/weight tiles ride partition-packed multi-tile DMAs ordered by
  need-time, with the FIRST tiles split across all four engine queues so
  phase 1 starts as early as possible.

All matmuls run in bf16 with f32 PSUM accumulation; softmax runs without
max-subtraction with the denominator computed for free via a ones-column
appended to V. Output is stored bf16 and widened to f32 on the host.
"""
import math
import sys
import types

import ml_dtypes
import numpy as np

if "/opt/trn_rl_repo" not in sys.path:
    sys.path.insert(0, "/opt/trn_rl_repo")


def _install_ntff_hook():
    """Register the axon NTFF profile hook (missing antenv.axon_hooks shim)."""
    if "antenv.axon_hooks" in sys.modules:
        return
    mod = types.ModuleType("antenv.axon_hooks")
    _h = [None]
    mod.set_axon_ntff_profile_hook = lambda h: _h.__setitem__(0, h)
    mod.get_axon_ntff_profile_hook = lambda: _h[0]
    sys.modules["antenv.axon_hooks"] = mod
    try:
        import antenv
        antenv.axon_hooks = mod
        from trn_agent_boot.trn_boot import _ntff_profile_via_ctypes
        mod.set_axon_ntff_profile_hook(
            _ntff_profile_via_ctypes("/opt/axon/libaxon_pjrt.so")
        )
    except Exception:
        pass


_install_ntff_hook()

import concourse.bass_utils as _bass_utils
_bass_utils.upload_artifacts = lambda d: d  # no artifact bucket in this env

import concourse.bacc as bacc
import concourse.tile as tile
import concourse.mybir as mybir
from concourse.bass_utils import run_bass_kernel_spmd

BF16 = mybir.dt.bfloat16
F32 = mybir.dt.float32

B, S, H = 2, 2048, 2048
D = 128              # head dim
NHL = 4              # local Q heads per core
NT = 16              # 128-tiles along H / S / attn-dim
NQC = 4              # 512-wide q chunks
QC = 512
N_CORES = 8
SCALE = 1.0 / math.sqrt(D)
WARM_MMS = 16        # dummy warmup matmuls before real data lands

_CACHE = {}


def _build():
    if "nc" in _CACHE:
        return _CACHE["nc"]

    nc = bacc.Bacc("TRN2", target_bir_lowering=False, debug=False,
                   num_devices=N_CORES)

    # All inputs are pre-packed on the HOST into partition-major layouts:
    # row p holds tile-p rows of every 128-row tile back to back, so each
    # partition's data for a multi-tile load is one contiguous 8-16KB run
    # (1-2 DMA descriptors instead of 4-16 4KB ones — the descriptor fetch
    # (Q_I) otherwise costs as much engine time as the transfer itself).
    hid_ext = nc.dram_tensor("hidt", [128, NT * S], BF16, kind="ExternalInput")
    wq_ext = nc.dram_tensor("wq", [128, NT * NHL * D], BF16,
                            kind="ExternalInput")
    # wkv tiles ++ ident (cols 0-127) ++ bmask (cols 128-129), one DMA
    wkv_ext = nc.dram_tensor("wkv", [128, NT * 2 * D + 130], BF16,
                             kind="ExternalInput")
    wo_ext = nc.dram_tensor("wo", [128, NT * H], BF16, kind="ExternalInput")
    out_ext = nc.dram_tensor("out", [QC, H], BF16, kind="ExternalOutput")

    with tile.TileContext(nc) as tc:
        with tc.tile_pool(name="dram", bufs=1, space="DRAM") as dram, \
             tc.tile_pool(name="persist", bufs=1) as per, \
             tc.tile_pool(name="attpool", bufs=4) as atp, \
             tc.tile_pool(name="work", bufs=3) as wk_pool, \
             tc.tile_pool(name="pairs", bufs=2, space="PSUM") as psA, \
             tc.tile_pool(name="accs", bufs=2, space="PSUM") as psB, \
             tc.tile_pool(name="psqp", bufs=1, space="PSUM") as psC, \
             tc.tile_pool(name="tpp", bufs=1, space="PSUM") as psD:

            identb = per.tile([128, 130], BF16, name="identb_sb")
            bmask = per.tile([128, 2], F32, name="bmask_sb")
            dummy = per.tile([128, 640], BF16, name="dummy_sb")

            qT = [per.tile([128, S], BF16, name=f"qT{h}") for h in range(NHL)]
            kT = per.tile([128, S], BF16, name="kT")
            v_aug = [per.tile([128, D + 1], BF16, name=f"vaug{st}")
                     for st in range(NT)]
            # per-head A2A bounce buffers
            send = [dram.tile([N_CORES, 128, QC], BF16, name=f"send{h}")
                    for h in range(NHL)]
            recv = [dram.tile([N_CORES, 128, QC], BF16, name=f"recv{h}")
                    for h in range(NHL)]
            gathered = [per.tile([128, QC], BF16, name=f"gat{at}")
                        for at in range(NT)]

            # warmup operand (ZERO so fillers can add into open accumulation
            # groups harmlessly), then ones columns of v_aug
            nc.gpsimd.memset(dummy[:], 0.0)
            for st in range(NT):
                nc.gpsimd.memset(v_aug[st][:, D:], 1.0)

            # ---- PE warmup: dep-free matmuls on the memset tile keep the
            # PE busy (and the p-state ramp warm) while inputs stream in.
            warm = psD.tile([128, QC], F32, tag="tp", name="warm")
            for i in range(WARM_MMS):
                nc.tensor.matmul(warm[:], lhsT=dummy[:, 0:128],
                                 rhs=dummy[:, 128:640], start=True, stop=True)

            last_copy = [None]
            pend_flush = [None]

            def attention(h, proj=None, defer_tail=False):
                # Scores are emitted in PAIRS (two N=512 matmuls into one
                # [128,1024] two-bank tile) consumed by ONE wide exp; the PV
                # block for pair p is emitted during pair p+1 (after its exp)
                # so the PE never waits on the current exp. The transposed
                # flush of a q-chunk is deferred into the next chunk as in
                # the baseline.
                #
                # `proj`, if given, is (next_head, wq_sb, hidT): one qT
                # projection matmul is emitted per kt step so the projection
                # rides inside attention instead of serializing at the head
                # boundary (it also keeps the PE ahead of ACT).
                def coll():
                    nc.gpsimd.collective_compute(
                        "AllToAll", mybir.AluOpType.bypass,
                        replica_groups=[list(range(N_CORES))],
                        ins=[send[h][:]], outs=[recv[h][:]],
                    )

                def flush(qc, obs, with_coll=False):
                    tp = psD.tile([128, QC], F32, tag="tp",
                                  name=f"tpo_{h}_{qc}")
                    for qs in range(4):
                        nc.tensor.matmul(tp[:, qs * 128:(qs + 1) * 128],
                                         lhsT=obs[qs][:], rhs=identb[:, 0:128],
                                         start=True, stop=True)
                    at_h = atp.tile([128, QC], BF16, tag="attnT",
                                    name=f"attnT_{h}_{qc}")
                    last_copy[0] = nc.vector.tensor_copy(at_h[:], tp[:])
                    # A2A sends for this q-chunk (dest rank qc of both halves)
                    nc.sync.dma_start(send[h][qc], at_h[:])
                    nc.sync.dma_start(send[h][4 + qc], at_h[:])
                    if with_coll:
                        coll()

                for qc in range(NQC):
                    cs = slice(qc * QC, (qc + 1) * QC)
                    # packed accumulators: accs 0-2 in one bank, acc 3 in
                    # the next (ones-column denominators in col D of each)
                    acc012 = psB.tile([128, 3 * (D + 1)], F32, tag="accs",
                                      name=f"acc012_{h}_{qc}")
                    acc3 = psB.tile([128, D + 1], F32, tag="accs",
                                    name=f"acc3_{h}_{qc}")

                    def acc_sl(qs):
                        if qs < 3:
                            return acc012[:, qs * (D + 1):(qs + 1) * (D + 1)]
                        return acc3[:, 0:D + 1]

                    if proj is not None:
                        ph, wq_sb, hidT = proj
                        psq = psC.tile([128, QC], F32, tag="psq",
                                       name=f"psq_{ph}_{qc}")
                    pending_pv = [None]

                    def emit_pv(pair, pt2):
                        for half in range(2):
                            kt = 2 * pair + half
                            po = half * QC
                            for qs in range(4):
                                nc.tensor.matmul(
                                    acc_sl(qs),
                                    lhsT=pt2[:, po + qs * 128:
                                             po + (qs + 1) * 128],
                                    rhs=v_aug[kt][:],
                                    start=(kt == 0 and qs == 0) or
                                          (kt == 0 and qs == 3),
                                    stop=(kt == NT - 1 and qs == 2) or
                                         (kt == NT - 1 and qs == 3),
                                    skip_group_check=True)

                    for pair in range(NT // 2):
                        kt0, kt1 = 2 * pair, 2 * pair + 1
                        sc2 = psA.tile([128, 2 * QC], F32, tag="pairs",
                                       name=f"sc_{h}_{qc}_{pair}")
                        nc.tensor.matmul(
                            sc2[:, 0:QC],
                            lhsT=kT[:, kt0 * 128:(kt0 + 1) * 128],
                            rhs=qT[h][:, cs], start=True, stop=True)
                        nc.tensor.matmul(
                            sc2[:, QC:2 * QC],
                            lhsT=kT[:, kt1 * 128:(kt1 + 1) * 128],
                            rhs=qT[h][:, cs], start=True, stop=True)
                        pt2 = wk_pool.tile([128, 2 * QC], BF16, tag="pt",
                                           name=f"pt_{h}_{qc}_{pair}")
                        nc.scalar.activation(
                            pt2[:], sc2[:], mybir.ActivationFunctionType.Exp,
                            scale=SCALE)
                        if pair == 1 and pend_flush[0] is not None:
                            f = pend_flush[0]
                            pend_flush[0] = None
                            f()
                        if pending_pv[0] is not None:
                            emit_pv(pair - 1, pending_pv[0])
                            if proj is None:
                                # head 3 is ACT-bound: a zero-adding filler
                                # into the open acc bank bridges the ~230ns
                                # per-pair PE idle so the p-state holds.
                                nc.tensor.matmul(
                                    acc012[:, 0:500], lhsT=dummy[:, 0:128],
                                    rhs=dummy[:, 128:628],
                                    start=False, stop=False,
                                    skip_group_check=True)
                        if proj is not None:
                            nc.tensor.matmul(
                                psq[:], lhsT=wq_sb[kt0][:, ph * D:(ph + 1) * D],
                                rhs=hidT[kt0][:, cs],
                                start=(kt0 == 0), stop=False)
                            nc.tensor.matmul(
                                psq[:], lhsT=wq_sb[kt1][:, ph * D:(ph + 1) * D],
                                rhs=hidT[kt1][:, cs],
                                start=False, stop=(kt1 == NT - 1))
                        pending_pv[0] = pt2
                    emit_pv(NT // 2 - 1, pending_pv[0])

                    # normalize (frees the acc banks for the next chunk)
                    obs = []
                    for qs in range(4):
                        asl = acc_sl(qs)
                        rec = wk_pool.tile([128, 1], F32, tag="rec",
                                           name=f"rec_{h}_{qc}_{qs}")
                        nc.vector.reciprocal(rec[:], asl[:, D:D + 1])
                        ob = wk_pool.tile([128, D], BF16, tag="ob", bufs=8,
                                          name=f"ob_{h}_{qc}_{qs}")
                        nc.vector.tensor_scalar_mul(ob[:], asl[:, :D], rec[:])
                        obs.append(ob)
                    if proj is not None:
                        # psq is single-buffered: drain on ACT (which has
                        # slack between the last exp of this chunk and the
                        # first of the next) so the next chunk's first proj
                        # matmul never waits on the DVE normalize backlog.
                        nc.scalar.copy(qT[ph][:, cs], psq[:])
                    pend_flush[0] = (
                        lambda q=qc, o=obs, wc=(qc == NQC - 1): flush(q, o, wc))
                if not defer_tail:
                    # flush the last chunk and issue the A2A now (the next
                    # phase needs the gpsimd queue or there is no next head)
                    f = pend_flush[0]
                    pend_flush[0] = None
                    f()

            rtiles = {}

            def recv_load(h, eng=None):
                # recv DMAs for head h, emitted right after the NEXT head's
                # collective issue so they sit early in the gpsimd stream
                # (head h's collective has finished by then -> no stall).
                # One partition-packed DMA per batch half (4 ranks each).
                lo = wk_pool.tile([128, 4 * QC], BF16, tag="rlo", bufs=3,
                                  name=f"rlo_{h}")
                hi = wk_pool.tile([128, 4 * QC], BF16, tag="rhi", bufs=3,
                                  name=f"rhi_{h}")
                e = eng if eng is not None else nc.gpsimd
                e.dma_start(out=lo[:].rearrange("p (b c) -> p b c", c=QC),
                            in_=recv[h][0:4].transpose([1, 0, 2]))
                e.dma_start(out=hi[:].rearrange("p (b c) -> p b c", c=QC),
                            in_=recv[h][4:8].transpose([1, 0, 2]))
                for gp in range(4):
                    cs = slice(gp * QC, (gp + 1) * QC)
                    rtiles[(h, gp)] = (lo[:, cs], hi[:, cs])

            def combine(h, after=None):
                # receive-side batch mask:
                # gathered[4*gp + h] = recv_lo*m0 + recv_hi*m1
                # Explicitly ordered after `after` (default: the last
                # attention's DVE work) so the collective wait can never
                # stall the DVE stream (Tile's cost model underestimates
                # the collective and would otherwise hoist these).
                e = nc.vector
                anchor = after if after is not None else last_copy[0]
                for gp in range(4):
                    lo, hi = rtiles[(h, gp)]
                    mul = e.tensor_scalar_mul(hi, hi, bmask[:, 1:2])
                    if anchor is not None:
                        tile.add_dep_helper(
                            mul.ins, anchor.ins, sync=False,
                            reason="combine ordered behind compute stream")
                    e.scalar_tensor_tensor(
                        gathered[4 * gp + h][:], lo, bmask[:, 0:1],
                        hi, mybir.AluOpType.mult, mybir.AluOpType.add)

            with tc.tile_pool(name="projpool", bufs=1) as pp:

                # Partition-packed input loads: one DMA fills several
                # 128-row tiles (row -> partition, tile -> column block).
                # Early tiles ride small DMAs split across ALL queues so
                # the kT/q0 accumulation starts as early as possible.
                class _Pack:
                    """tile-like view of column block [c0, c0+width) of t"""
                    def __init__(self, t, c0):
                        self.t, self.c0 = t, c0

                    def __getitem__(self, idx):
                        p, c = idx
                        return self.t[p, self.c0 + c.start:self.c0 + c.stop]

                def _dma(q, out, in_):
                    return q.dma_start(out=out, in_=in_)

                # Partition-major loads: plain 2D DMAs with one 8-16KB
                # descriptor per partition. Each dma_start costs ~2.1us of
                # queue latency on top of its transfer, so hidT rides in six
                # asymmetric groups (2/4/2 tiles per queue) — small leading
                # groups for an early start, big middle ones for throughput.
                # wkv(+identb) and wq ride gpsimd ahead of their need-times.
                wkv_t = pp.tile([128, NT * 2 * D + 130], BF16, name="wkv_all")
                nc.gpsimd.dma_start(wkv_t[:], wkv_ext[:])
                wkv_views = [_Pack(wkv_t, ht * 2 * D) for ht in range(NT)]

                GROUPS = [(nc.sync, 0, 2), (nc.scalar, 2, 2),
                          (nc.sync, 4, 4), (nc.scalar, 8, 4),
                          (nc.sync, 12, 2), (nc.scalar, 14, 2)]
                hidT = [None] * NT
                for gi, (q, first, n) in enumerate(GROUPS):
                    t = pp.tile([128, n * S], BF16, name=f"hidT_g{gi}")
                    _dma(q, t[:], hid_ext[:, first * S:(first + n) * S])
                    for i in range(n):
                        hidT[first + i] = _Pack(t, i * S)
                    if gi == 0:
                        wq_t1 = pp.tile([128, 8 * NHL * D], BF16, name="wq_a")
                        nc.gpsimd.dma_start(wq_t1[:],
                                            wq_ext[:, 0:8 * NHL * D])
                    if gi == 1:
                        wq_t2 = pp.tile([128, 8 * NHL * D], BF16, name="wq_b")
                        nc.gpsimd.dma_start(wq_t2[:],
                                            wq_ext[:, 8 * NHL * D:])
                wq_sb = [_Pack(wq_t1, ht * NHL * D) for ht in range(8)]
                wq_sb += [_Pack(wq_t2, ht * NHL * D) for ht in range(8)]
                wk_sb = [wkv_views[ht][:, slice(0, D)] for ht in range(NT)]
                wv_sb = [wkv_views[ht][:, slice(D, 2 * D)] for ht in range(NT)]
                nc.vector.tensor_copy(identb[:], wkv_t[:, NT * 2 * D:])
                nc.vector.tensor_copy(bmask[:], identb[:, 128:130])

                # ---- phase 1: kT and head-0 qT accumulate per arriving
                # hidT tile (PE-paced ~1.7us/tile, chasing the DMA stream),
                # then the v projection runs over the resident tiles.
                # PSUM: kT in the two [128,1024] pair slots (2 banks each),
                # q0 in the acc/psq/tp banks.
                kt_pair = [psA.tile([128, 2 * QC], F32, tag="pairs",
                                    name=f"psk{i}") for i in range(2)]
                psk = [kt_pair[i // 2][:, (i % 2) * QC:(i % 2 + 1) * QC]
                       for i in range(4)]
                q0a = psB.tile([128, QC], F32, tag="accs", name="psq0_0")
                q0b = psB.tile([128, QC], F32, tag="accs", name="psq0_1")
                q0c = psC.tile([128, QC], F32, tag="psq", name="psq0_2")
                q0d = psD.tile([128, QC], F32, tag="tp", name="psq0_3")
                psq0 = [q0a[:], q0b[:], q0c[:], q0d[:]]
                def q0_mms(ht):
                    for qc in range(4):
                        nc.tensor.matmul(
                            psq0[qc], lhsT=wq_sb[ht][:, 0:D],
                            rhs=hidT[ht][:, qc * QC:(qc + 1) * QC],
                            start=(ht == 0), stop=(ht == NT - 1))

                # q0 lags kT by two tiles so the first wq pack has time to
                # land; zero-adding fillers into the open kT groups cover
                # DMA-arrival jitter so the PE p-state ramp never resets.
                for ht in range(NT):
                    for sb in range(4):
                        nc.tensor.matmul(
                            psk[sb], lhsT=wk_sb[ht],
                            rhs=hidT[ht][:, sb * QC:(sb + 1) * QC],
                            start=(ht == 0), stop=(ht == NT - 1))
                    if ht >= 2:
                        q0_mms(ht - 2)
                    if ht % 2 == 1 and 0 < ht < NT - 1:
                        for f in range(2):
                            nc.tensor.matmul(
                                psk[f % 4], lhsT=dummy[:, 0:128],
                                rhs=dummy[:, 128:640],
                                start=False, stop=False,
                                skip_group_check=True)
                q0_mms(NT - 2)
                q0_mms(NT - 1)
                # drain to SBUF (split across DVE and ACT queues); kT first
                # so the v projection can start in the freed pair slots.
                def _copy(eng, dst, src):
                    if eng is nc.vector:
                        eng.tensor_copy(dst, src)
                    else:
                        eng.copy(dst, src)

                for sb in range(4):
                    cs = slice(sb * QC, (sb + 1) * QC)
                    _copy(nc.vector if sb % 2 == 0 else nc.scalar,
                          kT[:, cs], psk[sb])
                for sb in range(4):
                    cs = slice(sb * QC, (sb + 1) * QC)
                    _copy(nc.scalar if sb % 2 == 0 else nc.vector,
                          qT[0][:, cs], psq0[sb])

                # v projection: one packed accumulation group per bank
                # (4 slices x 16 ht each), no sequential rounds.
                v_pair = [psA.tile([128, 2 * QC], F32, tag="pairs",
                                   name=f"psv{i}") for i in range(2)]
                psv = [v_pair[i // 2][:, (i % 2) * QC:(i % 2 + 1) * QC]
                       for i in range(4)]
                for ht in range(NT):
                    for sb2 in range(4):
                        for sl in range(4):
                            st = 4 * sb2 + sl
                            nc.tensor.matmul(
                                psv[sb2][:, sl * 128:(sl + 1) * 128],
                                lhsT=hidT[ht][:, st * 128:(st + 1) * 128],
                                rhs=wv_sb[ht],
                                start=(ht == 0 and sl == 0),
                                stop=(ht == NT - 1 and sl == 3),
                                skip_group_check=True)
                for st in range(NT):
                    sb2, sl = divmod(st, 4)
                    _copy(nc.vector if st % 2 == 0 else nc.scalar,
                          v_aug[st][:, :D],
                          psv[sb2][:, sl * 128:(sl + 1) * 128])
                for h in range(NHL - 1):
                    # heads 0 and 1 defer their last flush + A2A issue into
                    # the next head's pipeline (kills the head-boundary PE
                    # stall); head 2 must issue before the wo loads queue up
                    # on gpsimd.
                    attention(h, proj=(h + 1, wq_sb, hidT),
                              defer_tail=(h < 2))
                    if h >= 1:
                        recv_load(h - 1)
                    if h >= 2:
                        # head h-2's A2A finished a full head ago: its
                        # combine can never stall the DVE here
                        combine(h - 2)

            # projpool closed: hidT/wq freed; wo loads reuse that space and
            # overlap the last head's attention.
            with tc.tile_pool(name="late", bufs=1) as lp:
                # wo on the gpsimd ring: the sync ring must stay free for
                # the last head's sends (a queued wo load would delay its
                # collective by up to 26us). Partition-packed, 4 tiles/DMA.
                wo_sb = []
                for gi in range(4):
                    t = lp.tile([128, 4 * H], BF16, name=f"wo_p{gi}")
                    nc.gpsimd.dma_start(
                        t[:], wo_ext[:, gi * 4 * H:(gi + 1) * 4 * H])
                    wo_sb += [_Pack(t, i * H) for i in range(4)]

                # head 1's A2A completed during head 2's attention, so its
                # combine can sit in the DVE stream during head 3
                combine(NHL - 3)
                attention(NHL - 1)
                recv_load(NHL - 2)
                combine(NHL - 2)

                # ---- phase 5, pass A: accumulate heads 0-2 (12 of 16
                # steps) for all 16 output groups into SBUF partials. This
                # is ~42us of PE work with no dependence on the last
                # head's AllToAll, so the collective is fully hidden.
                # Steps are emitted in LOCKSTEP across the 8 in-flight
                # banks (four [128,1024] tiles, two groups each); drains
                # are PAIRED [128,1024] copies.
                ats012 = [4 * gp + h for h in range(NHL - 1) for gp in range(4)]
                ats3 = [4 * gp + (NHL - 1) for gp in range(4)]
                partial = [lp.tile([128, H], F32, name=f"par{st}")
                           for st in range(4)]
                lastA = [None]
                for batch in range(2):
                    t2a = psA.tile([128, 2 * QC], F32, tag="pairs",
                                   name=f"psoA_{batch}_a")
                    t2b = psA.tile([128, 2 * QC], F32, tag="pairs",
                                   name=f"psoA_{batch}_b")
                    pss = [t2a[:, 0:QC], t2a[:, QC:2 * QC],
                           t2b[:, 0:QC], t2b[:, QC:2 * QC],
                           psB.tile([128, QC], F32, tag="accs",
                                    name=f"psoA_{batch}_4")[:],
                           psB.tile([128, QC], F32, tag="accs",
                                    name=f"psoA_{batch}_5")[:],
                           psC.tile([128, QC], F32, tag="psq",
                                    name=f"psoA_{batch}_6")[:],
                           psD.tile([128, QC], F32, tag="tp",
                                    name=f"psoA_{batch}_7")[:]]
                    for i, at in enumerate(ats012):
                        for j in range(8):
                            g16 = batch * 8 + j
                            st, hc = divmod(g16, 4)
                            ss = slice(st * 128, (st + 1) * 128)
                            nc.tensor.matmul(
                                pss[j], lhsT=gathered[at][:, ss],
                                rhs=wo_sb[at][:, hc * QC:(hc + 1) * QC],
                                start=(i == 0), stop=(i == len(ats012) - 1))
                    # drains: the two psA tiles are contiguous (hc,hc+1)
                    # pieces of one partial row -> single wide copies.
                    st0 = (batch * 8) // 4
                    lastA[0] = nc.vector.tensor_copy(
                        partial[st0][:, 0:2 * QC], t2a[:])
                    nc.scalar.copy(partial[st0][:, 2 * QC:4 * QC], t2b[:])
                    for j in range(4, 8):
                        g16 = batch * 8 + j
                        st, hc = divmod(g16, 4)
                        dst = partial[st][:, hc * QC:(hc + 1) * QC]
                        if j % 2 == 0:
                            lastA[0] = nc.vector.tensor_copy(dst, pss[j])
                        else:
                            nc.scalar.copy(dst, pss[j])
                    if batch == 0:
                        # head-3 recvs on the sync queue (it waits on the
                        # collective there, long after the sends went out);
                        # combine(3) slots between pass A's two batches —
                        # anchored after batch 0's last DVE copy so the
                        # collective wait can't starve batch-0 bank
                        # recycling, while head 3's gather completes before
                        # pass A ends and pass B starts with no bubble.
                        recv_load(NHL - 1, eng=nc.sync)
                        combine(NHL - 1, after=lastA[0])

                # ---- pass B: add head 3 and merge with the partials; store
                # each 512-wide piece as soon as it's ready.
                # keep the out stores off gpsimd: its end-of-kernel drain is
                # slow, so letting it go idle early overlaps the drain with
                # pass B instead of the teardown
                oqueues = [nc.sync, nc.scalar]
                bpools = [(psA, "pairs"), (psB, "accs"),
                          (psC, "psq"), (psD, "tp")]
                for st in range(4):
                    ss = slice(st * 128, (st + 1) * 128)
                    for hc in range(4):
                        g16 = st * 4 + hc
                        pool, tg = bpools[g16 % 4]
                        if pool is psA:
                            t2 = psA.tile([128, 2 * QC], F32, tag="pairs",
                                          name=f"psoB_{st}_{hc}")
                            ps = t2[:, 0:QC]
                        else:
                            ps = pool.tile([128, QC], F32, tag=tg,
                                           name=f"psoB_{st}_{hc}")[:]
                        for i, at in enumerate(ats3):
                            nc.tensor.matmul(
                                ps, lhsT=gathered[at][:, ss],
                                rhs=wo_sb[at][:, hc * QC:(hc + 1) * QC],
                                start=(i == 0), stop=(i == len(ats3) - 1))
                        ob = wk_pool.tile([128, QC], BF16, tag="opiece",
                                          bufs=4, name=f"op_{st}_{hc}")
                        nc.vector.scalar_tensor_tensor(
                            ob[:], ps, 1.0,
                            partial[st][:, hc * QC:(hc + 1) * QC],
                            mybir.AluOpType.mult, mybir.AluOpType.add)
                        oqueues[g16 % 2].dma_start(
                            out_ext[ss, hc * QC:(hc + 1) * QC], ob[:])

    nc.compile()
    _CACHE["nc"] = nc
    return nc


def _pmajor(a):
    """[NT*128, W] row-tiled matrix -> [128, NT*W] partition-major layout."""
    nt = a.shape[0] // 128
    return np.ascontiguousarray(
        a.reshape(nt, 128, a.shape[1]).transpose(1, 0, 2).reshape(128, -1))


def _make_in_maps(hidden_states, w_q, w_k, w_v, w_o):
    bf16 = ml_dtypes.bfloat16
    hidt_bf = [_pmajor(np.ascontiguousarray(hidden_states[b].T).astype(bf16))
               for b in range(B)]
    wq_bf = w_q.astype(bf16)
    wk_bf = w_k.astype(bf16)
    wv_bf = w_v.astype(bf16)
    wo_bf = _pmajor(w_o.astype(bf16))
    in_maps = []
    for c in range(N_CORES):
        b, g = c // 4, c % 4
        m0 = 1.0 if b == 0 else 0.0
        identb = np.zeros((128, 130), dtype=bf16)
        identb[:, :128] = np.eye(128, dtype=bf16)
        identb[:, 128] = bf16(m0)
        identb[:, 129] = bf16(1.0 - m0)
        wkv = np.concatenate(
            [wk_bf[:, g * D:(g + 1) * D], wv_bf[:, g * D:(g + 1) * D]],
            axis=1)
        wkvid = np.concatenate([_pmajor(np.ascontiguousarray(wkv)), identb],
                               axis=1)
        in_maps.append({
            "hidt": hidt_bf[b],
            "wq": _pmajor(np.ascontiguousarray(
                wq_bf[:, g * NHL * D:(g + 1) * NHL * D])),
            "wkv": np.ascontiguousarray(wkvid),
            "wo": wo_bf,
        })
    return in_maps


def _run(hidden_states, w_q, w_k, w_v, w_o, trace=False):
    nc = _build()
    in_maps = _make_in_maps(hidden_states, w_q, w_k, w_v, w_o)
    res = run_bass_kernel_spmd(nc, in_maps, list(range(N_CORES)), trace=trace)
    out = np.empty((B, S, H), np.float32)
    for c in range(N_CORES):
        b, q = c // 4, c % 4
        out[b, q * QC:(q + 1) * QC, :] = res.results[c]["out"].astype(np.float32)
    return out, res


def kernel(hidden_states, position_ids=None, w_q=None, w_k=None, w_v=None,
           w_o=None):
    hidden_states = np.asarray(hidden_states, dtype=np.float32)
    w_q = np.asarray(w_q, dtype=np.float32)
    w_k = np.asarray(w_k, dtype=np.float32)
    w_v = np.asarray(w_v, dtype=np.float32)
    w_o = np.asarray(w_o, dtype=np.float32)
    out, _ = _run(hidden_states, w_q, w_k, w_v, w_o, trace=False)
    return out
